# revision 42
# baseline (speedup 1.0000x reference)
"""Bass/Trainium2 kernel for attention-LSTM decoder (nn_Attention_49289044688898).

Data-parallel over batch: 512 rows -> 8 NeuronCores x 64 rows. Weights replicated.
Within a core, 64 rows = 2 groups of 32 for the attention; LSTM/q/probs joint.

Schedule v1 (post-baseline): denominator computed from est via DVE
tl-reduce + ones-matmul (x32 redundancy folded into the reciprocal scale),
killing the alphaT spread DMAs; ablk block-diag written DIRECTLY from est
with one DMA per b-parity; ctx accumulation split across two PSUM banks
(even/odd kt interleave) to avoid same-bank matmul serialization; gates
packed as FI=[f,i] / GO=[o,g] with one [128,512] ACT op per bank (per-
partition scale vector selects tanh vs scaled-tanh) + DVE fixups; the
attention window emits (g0,k),(g1,k) interleaved with exp(g) explicitly
slotted between the last two tanh chunks.

Per step s (26 steps):
  hT  = transpose(h)                        (PE, 4 transpose-mm)
  qT  = WhT-chunks @ hT                     (PE, 16 mm N=64, k-accum)
  probs(s-1) = hT-mm @ Wgen + bg            (PE + DVE, during tanh window)
  z-partial: onehot@Ko' + h@R               (PE, during tanh window)
  per (k, g): th = tanh(HprojT + qT)        (DVE add + ACT tanh)
              e-mms into PSUM quadrants     (PE)
  per g: est = exp(e-PSUM)                  (ACT)
         ablk direct scatter from est      (2 DMA: sync+gpsimd)
         red[p,b] = sum_tl est              (DVE 3D reduce)
         32*denom = red^T @ ones            (PE, N=1) -> reciprocal (DVE)
         ctx: E/O bank-alternating mms      (PE), (E+O)*rcp*32 -> ctx_sb (DVE)
  xTc = PE-transpose(ctx)
  z  += xTc @ Kc                            (PE)
  gates: 2 packed ACT ops + fixups + c/h    (ACT+DVE)
Layouts:
  attention world: [128 part = h_lo, 4 h_hi, 64 t, 32 b]
  context world:   [128 part = (b%2)*64 + t, 16 kt=b//2, 512 c]
  LSTM world:      [64 part = b, free]; FI bank=[f;i], GO bank=[o;g]
"""

import numpy as np
import ml_dtypes
from contextlib import ExitStack

B, T, C, H, NCC, S = 512, 64, 512, 512, 96, 26
NCORES = 8
BS = B // NCORES          # 64 batch rows per core
NG = 2                    # groups per core
GB = BS // NG             # 32 rows per group
BF = ml_dtypes.bfloat16

_CACHE = {}


def build_bass():
    import concourse.bass as bass
    import concourse.bacc as bacc
    import concourse.tile as tile
    import concourse.mybir as mybir

    f32 = mybir.dt.float32
    bf16 = mybir.dt.bfloat16
    f8 = mybir.dt.float8e4
    DR = mybir.MatmulPerfMode.DoubleRow
    AF = mybir.ActivationFunctionType
    AX = mybir.AxisListType
    AL = mybir.AluOpType

    nc = bacc.Bacc("TRN2", target_bir_lowering=False)

    # ---- DRAM I/O ----
    bHT_d = nc.dram_tensor("bHT", [NG, C, T, GB], bf16, kind="ExternalInput")
    bHc_d = nc.dram_tensor("bHc", [NG, GB // 2, 128, C], bf16, kind="ExternalInput")
    wi_d = nc.dram_tensor("wi", [C, H], bf16, kind="ExternalInput")
    wh_d = nc.dram_tensor("wh", [H, H], bf16, kind="ExternalInput")
    bh_d = nc.dram_tensor("bh", [128, 4], f32, kind="ExternalInput")
    ws_d = nc.dram_tensor("ws", [128, 4, 32], bf16, kind="ExternalInput")
    kc_d = nc.dram_tensor("kc", [C, 4 * H], bf16, kind="ExternalInput")
    rr_d = nc.dram_tensor("rr", [H, 4 * H], bf16, kind="ExternalInput")
    ko_d = nc.dram_tensor("ko", [NCC, 4 * H], bf16, kind="ExternalInput")
    oh_d = nc.dram_tensor("oh", [NCC, S, BS], bf16, kind="ExternalInput")
    wg_d = nc.dram_tensor("wg", [H, NCC], bf16, kind="ExternalInput")
    bg_d = nc.dram_tensor("bg", [BS, NCC], f32, kind="ExternalInput")
    id_d = nc.dram_tensor("ident", [128, 128], bf16, kind="ExternalInput")
    out_d = nc.dram_tensor("out", [BS, S, NCC], f32, kind="ExternalOutput")

    NCH = T * GB // 512  # 4 (t,b)-chunks of 512 per group

    with tile.TileContext(nc) as tc, ExitStack() as ctx:
        big = ctx.enter_context(tc.tile_pool(name="big", bufs=1))
        wpool = ctx.enter_context(tc.tile_pool(name="wpool", bufs=1))
        small = ctx.enter_context(tc.tile_pool(name="small", bufs=2))
        tiny = ctx.enter_context(tc.tile_pool(name="tiny", bufs=4))
        gates = ctx.enter_context(tc.tile_pool(name="gates", bufs=4))
        state = ctx.enter_context(tc.tile_pool(name="state", bufs=2))
        # PSUM pools (8 banks total):
        #   pz:  FI + GO gate accumulators  [128,512] x2     = 2 banks
        #   pep: e quadrant accumulator     [128,512] bufs=2 = 2 banks
        #   pcp: ctx E/O accumulators       [128,512] x2     = 2 banks
        #   ptp: bf16 PE-transpose outs     [128,256] bufs=1 = 1 bank
        #   psm: qT/probs/sums f32 mm outs  [128,256] bufs=1 = 1 bank
        pz = ctx.enter_context(tc.tile_pool(name="pz", bufs=1, space="PSUM"))
        pep = ctx.enter_context(tc.tile_pool(name="pep", bufs=2, space="PSUM"))
        pcp = ctx.enter_context(tc.tile_pool(name="pcp", bufs=1, space="PSUM"))
        ptp = ctx.enter_context(tc.tile_pool(name="ptp", bufs=1, space="PSUM"))
        psm = ctx.enter_context(tc.tile_pool(name="psm", bufs=1, space="PSUM"))

        dma = nc.sync
        import concourse.bass as _b

        # ---- load weights / big tensors ----
        bHc = [big.tile([128, GB // 2, C], bf16, tag=f"bHc{g}", name=f"bHc{g}")
               for g in range(NG)]
        for g in range(NG):
            dma.dma_start(out=bHc[g], in_=bHc_d[g].rearrange("k p c -> p k c"))
        # batch_H^T (prolog only; shares slots with tanh buffers)
        bHT = [big.tile([128, 4, T * GB], bf16, tag=f"th{g}", name=f"bHT{g}")
               for g in range(NG)]
        for g in range(NG):
            dma.dma_start(
                out=bHT[g],
                in_=bHT_d[g].rearrange("(ch cl) t b -> cl ch (t b)", cl=128))

        wi = wpool.tile([128, 4, H], bf16, tag="wi")
        dma.dma_start(out=wi, in_=wi_d[:].rearrange("(ch cl) h -> cl ch h", cl=128))
        wh = wpool.tile([128, 4, H], bf16, tag="wh")
        dma.dma_start(out=wh, in_=wh_d[:].rearrange("(hh hl) h -> hl hh h", hl=128))
        bh = wpool.tile([128, 4], f32, tag="bh")
        dma.dma_start(out=bh, in_=bh_d[:])
        ws = wpool.tile([128, 4, 32], bf16, tag="ws")
        dma.dma_start(out=ws, in_=ws_d[:])
        kc = wpool.tile([128, 4, 4 * H], bf16, tag="kc")
        dma.dma_start(out=kc, in_=kc_d[:].rearrange("(kh kl) n -> kl kh n", kl=128))
        rr = wpool.tile([128, 4, 4 * H], bf16, tag="rr")
        dma.dma_start(out=rr, in_=rr_d[:].rearrange("(kh kl) n -> kl kh n", kl=128))
        ko = wpool.tile([NCC, 4 * H], bf16, tag="ko")
        dma.dma_start(out=ko, in_=ko_d[:])
        oh = wpool.tile([NCC, S, BS], bf16, tag="oh")
        dma.dma_start(out=oh, in_=oh_d[:])
        wg = wpool.tile([128, 4, NCC], bf16, tag="wg")
        dma.dma_start(out=wg, in_=wg_d[:].rearrange("(hh hl) n -> hl hh n", hl=128))
        bg = wpool.tile([BS, NCC], f32, tag="bg")
        dma.dma_start(out=bg, in_=bg_d[:])
        ident = wpool.tile([128, 128], bf16, tag="ident")
        dma.dma_start(out=ident, in_=id_d[:])
        # 1/32 folds away the x32 copy-redundancy of the est row-blocks, so
        # the ones-matmul over all 128 partitions yields the denominator
        ones32 = wpool.tile([128, 1], bf16, tag="ones32")
        nc.vector.memset(ones32, 1.0 / 32.0)
        # per-partition activation scale for the packed GO bank: o rows
        # (0-63) get 0.5 (sigmoid-via-tanh), g rows (64-127) get 1.0
        ogsc = wpool.tile([128, 1], f32, tag="ogsc")
        nc.vector.memset(ogsc[0:64, :], 0.5)
        nc.vector.memset(ogsc[64:128, :], 1.0)
        pr_all = wpool.tile([BS, S, NCC], f32, tag="pr_all")

        # block-diag alpha holders (zeroed once)
        ablk = [wpool.tile([128, GB // 2, GB], bf16, tag=f"ablk{g}", name=f"ablk{g}")
                for g in range(NG)]
        for g in range(NG):
            nc.vector.memset(ablk[g], 0.0)

        # initial state (hT is never read at s=0: qT/z-rec are skipped)
        hT = [None]
        c_st = [state.tile([BS, H], f32, tag="c", name="c0")]
        nc.vector.memset(c_st[0], 0.0)
        hbf = [None]

        # ---- prolog: HprojT[g] = (batch_H @ Wi)^T + bh ----
        hprojT = [big.tile([128, 4, T * GB], bf16, tag=f"hp{g}", name=f"hp{g}")
                  for g in range(NG)]
        for g in range(NG):
            for m in range(4):
                for n in range(NCH):
                    ps = pz.tile([128, 512], f32, tag="FI" if g == 0 else "GO")
                    for k in range(4):
                        nc.tensor.matmul(
                            ps,
                            wi[:, k, m * 128:(m + 1) * 128],
                            bHT[g][:, k, n * 512:(n + 1) * 512],
                            start=(k == 0), stop=(k == 3),
                        )
                    if (m + n) % 2 == 0:
                        nc.scalar.activation(
                            out=hprojT[g][:, m, n * 512:(n + 1) * 512], in_=ps,
                            func=AF.Identity, bias=bh[:, m:m + 1], scale=1.0,
                        )
                    else:
                        nc.vector.tensor_scalar_add(
                            hprojT[g][:, m, n * 512:(n + 1) * 512], ps,
                            bh[:, m:m + 1])

        def bcast_t(ap2, nt=T):
            # [128, GB(b)] -> [128, nt(t, stride0), GB(b)]
            return _b.AP(tensor=ap2.tensor, offset=ap2.offset,
                         ap=[ap2.ap[0], [0, nt], ap2.ap[1]])

        gate_sl = {"f": 1, "i": 0, "g": 2, "o": 3}
        # gate -> (psum tag, row offset): f/i share FI bank; o/g share GO
        # bank with o LOW (rows 0-63) so h = sig(o)*tanh(c) stays aligned
        # at partitions 0-63 and t2 = sig(i)*tanh(g) reads both at 64-127.
        gate_loc = {"f": ("FI", 0), "i": ("FI", 64), "o": ("GO", 0), "g": ("GO", 64)}

        def emit_hT_transpose(s):
            # h_bf [64, 512] -> hT [128, 4, 64] via 4 PE transposes
            phT = ptp.tile([128, 256], bf16, tag="tp", name=f"phT_{s}")
            for m in range(4):
                nc.tensor.transpose(phT[:, m * 64:(m + 1) * 64],
                                    hbf[0][:, m * 128:(m + 1) * 128],
                                    ident[0:BS, 0:BS])
            hT[0] = state.tile([128, 4, BS], bf16, tag="hT", name=f"hT_{s}")
            nc.vector.tensor_copy(hT[0], phT)

        def emit_qT(s):
            # qT[h',b] = sum_h Wh[h,h'] hT[h,b]; m-outer so chunk m is
            # copied out as soon as its k-accumulation finishes.
            pqT = psm.tile([128, 256], f32, tag="pq", name=f"pqT_{s}")
            qT = small.tile([128, 4, BS], bf16, tag="qT", bufs=2, name=f"qT_{s}")
            for m in range(4):
                for k in range(4):
                    nc.tensor.matmul(pqT[:, m * 64:(m + 1) * 64],
                                     wh[:, k, m * 128:(m + 1) * 128],
                                     hT[0][:, k, :],
                                     start=(k == 0), stop=(k == 3))
                nc.vector.tensor_copy(qT[:, m, :], pqT[:, m * 64:(m + 1) * 64])
            return qT

        def emit_probs(sm1):
            # probs(sm1) = h(sm1) @ Wgen + bg, from hT
            pp = psm.tile([128, 256], f32, tag="pq", name=f"pp_{sm1}")
            for k in range(4):
                nc.tensor.matmul(pp[0:BS, 0:NCC], hT[0][:, k, :], wg[:, k, :],
                                 start=(k == 0), stop=(k == 3))
            nc.vector.tensor_add(pr_all[:, sm1, :], pp[0:BS, 0:NCC], bg)

        def emit_z_early(s, pzt):
            # onehot@Ko' (start) + h@R during the tanh window
            for gn in "figo":
                tag, ro = gate_loc[gn]
                zsl = slice(gate_sl[gn] * 512, (gate_sl[gn] + 1) * 512)
                nc.tensor.matmul(pzt[tag][ro:ro + 64, :], oh[:, s, :],
                                 ko[:, zsl], start=True, stop=False,
                                 tile_position=(0, ro))
            if s == 0:
                return  # h == 0: R contributes nothing
            for k in range(4):
                for gn in "figo":
                    tag, ro = gate_loc[gn]
                    zsl = slice(gate_sl[gn] * 512, (gate_sl[gn] + 1) * 512)
                    nc.tensor.matmul(pzt[tag][ro:ro + 64, :], hT[0][:, k, :],
                                     rr[:, k, zsl], start=False, stop=False,
                                     tile_position=(0, ro))

        def emit_chunk(s, g, k, qT, th_g, pe_g, split=False):
            # DVE add + ACT tanh + 4 e quadrant mms for (group g, chunk k).
            # split=True halves the tanh so the last e-mms (and thus exp)
            # start ~1us earlier — used for the tail-critical (g1, k3).
            gsl_b = slice(g * GB, (g + 1) * GB)
            src = hprojT[g]
            if qT is not None:
                nc.vector.tensor_add(
                    th_g[:, k, :].rearrange("p (t b) -> p t b", t=T),
                    hprojT[g][:, k, :].rearrange("p (t b) -> p t b", t=T),
                    bcast_t(qT[:, k, gsl_b]))
                src = th_g
            halves = 2 if split else 1
            hsz = (T * GB) // halves
            for hv in range(halves):
                sl = slice(hv * hsz, (hv + 1) * hsz)
                nc.scalar.activation(out=th_g[:, k, sl], in_=src[:, k, sl],
                                     func=AF.Tanh)
                for j in range(hv * NCH // halves, (hv + 1) * NCH // halves):
                    bp = 32 * j
                    nc.tensor.matmul(pe_g[bp:bp + 32, :], ws[:, k, :],
                                     th_g[:, k, j * 512:(j + 1) * 512],
                                     start=(k == 0), stop=(k == 3),
                                     tile_position=(0, bp))

        def emit_exp_dma(s, g, pe_g, split=False):
            # est[32j+r, tl*32+b] = exp(e(t=16j+tl, b)) for all r (the e-mm
            # wrote 32 identical copies per j). alphaT[t, b] built with ONE
            # row-spread DMA (contiguous 64B runs). The ablk block-diag is
            # NOT a DMA: per parity it is a same-partition free-dim restride
            # (stride 2 -> 34) plus a +64 partition shift for parity 1, so
            # parity 0 goes on idle GPSIMD and parity 1 on DVE (legal
            # cross-quadrant write for a 64-wide op).
            est = small.tile([128, 512], bf16, tag=f"est{g}", bufs=1,
                             name=f"est{g}_{s}")
            nc.scalar.activation(out=est, in_=pe_g, func=AF.Exp)
            ea = est[:]
            pe0 = ea.ap[0][0]
            alphaT = small.tile([T, GB], bf16, tag=f"alphaT{g}", bufs=2,
                                name=f"alphaT{g}_{s}")
            src = _b.AP(tensor=ea.tensor, offset=ea.offset,
                        ap=[[32 * pe0, 4], [32, 16], [1, 32]])
            # g1's spread DMA issues from the ACT queue right after its exp
            # (saves a cross-engine semaphore hop; ACT is idle post-window)
            (nc.scalar if g == 1 else dma).dma_start(out=alphaT[:, :], in_=src)
            emit_ablk_copy(g, alphaT, 0, nc.gpsimd)
            return est, alphaT

        def emit_ablk_copy(g, alphaT, par, eng):
            at = alphaT[:]
            ab = ablk[g][:]
            srcp = _b.AP(tensor=at.tensor,
                         offset=at.offset + par * at.ap[1][0],
                         ap=[[at.ap[0][0], T], [2 * at.ap[1][0], GB // 2]])
            dst = _b.AP(tensor=ab.tensor,
                        offset=ab.offset + par * (64 * ab.ap[0][0] + ab.ap[2][0]),
                        ap=[[ab.ap[0][0], T], [ab.ap[1][0] + 2 * ab.ap[2][0], GB // 2]])
            eng.tensor_copy(dst, srcp)

        def emit_red(s, g, est):
            # red[p, b] = sum_tl est[p, tl*32+b]; all 128 rows valid, so the
            # later ones-matmul over partitions yields 32x the denominator.
            red = small.tile([128, GB], bf16, tag=f"red{g}", bufs=2,
                             name=f"red{g}_{s}")
            with nc.allow_low_precision(reason="16-term positive sums; bf16 ok"):
                nc.vector.tensor_reduce(
                    out=red, in_=est[:].rearrange("p (tl b) -> p b tl", b=GB),
                    axis=AX.X, op=AL.add)
            return red

        def emit_post(s, g, red, ctx_sb, csum, psums, pcE, pcO, rcp_t):
            # 32*denominator via ones-matmul over partitions (per-group free
            # column of psums); ctx mms alternate E/O PSUM banks, both
            # groups on PSUM rows 0-31 (DoubleRow requires dst partition 0).
            ro = g * GB
            nc.tensor.matmul(psums[0:GB, g:g + 1], red, ones32,
                             start=True, stop=True, tile_position=(0, 0))
            nc.vector.reciprocal(rcp_t, psums[0:GB, g:g + 1])
            # kt-mms alternate E/O banks so accumulation pipelines
            banks = [pcE, pcO]
            for kt in range(GB // 2):
                nc.tensor.matmul(banks[kt % 2][0:GB, :], ablk[g][:, kt, :],
                                 bHc[g][:, kt, :],
                                 start=(kt < 2), stop=(kt >= GB // 2 - 2),
                                 tile_position=(0, 0))
            # (E + O) / denom with only one PSUM operand per DVE op; the
            # final op's out for g1 is a legal cross-quadrant 32-wide write
            nc.vector.tensor_scalar_mul(csum, pcE[0:GB, :], rcp_t)
            nc.vector.scalar_tensor_tensor(
                out=ctx_sb[ro:ro + GB, :], in0=pcO[0:GB, :],
                scalar=rcp_t, in1=csum,
                op0=AL.mult, op1=AL.add)

        def emit_ctxT(s, g, ctx_sb, pxT, xTc):
            # ctx rows of group g -> xTc[:, k, g*32:(g+1)*32]
            ro = g * GB
            for k in range(4):
                nc.tensor.transpose(pxT[g][:, k * GB:(k + 1) * GB],
                                    ctx_sb[ro:ro + GB, k * 128:(k + 1) * 128],
                                    ident[ro:ro + GB, ro:ro + GB])
            src = pxT[g][:, 0:128].rearrange("p (k b) -> p k b", k=4)
            nc.vector.tensor_copy(xTc[:, :, g * GB:(g + 1) * GB], src)

        def emit_z_late(s, pzt, xTc):
            for pair in ("fi", "og"):
                for k in range(4):
                    for gn in pair:
                        tag, ro = gate_loc[gn]
                        zsl = slice(gate_sl[gn] * 512, (gate_sl[gn] + 1) * 512)
                        nc.tensor.matmul(pzt[tag][ro:ro + 64, :], xTc[:, k, :],
                                         kc[:, k, zsl], start=False,
                                         stop=(k == 3), tile_position=(0, ro))

        def emit_gates(s, pzt):
            # FI bank = [f 0-63; i 64-127]: one tanh(x/2) op + DVE fixup.
            # GO bank = [o 0-63; g 64-127]: one tanh op with per-partition
            # scale (0.5 for o, 1.0 for g) + DVE fixup on the o half only.
            sfi = gates.tile([128, H], bf16, tag="sfi", bufs=2)
            nc.scalar.activation(out=sfi, in_=pzt["FI"], func=AF.Tanh, scale=0.5)
            nc.vector.tensor_scalar(out=sfi, in0=sfi, scalar1=0.5, scalar2=0.5,
                                    op0=AL.mult, op1=AL.add)
            # t1 first: it only needs the FI fixup, not sog
            t1 = gates.tile([BS, H], f32, tag="tmp", bufs=2)
            nc.vector.tensor_mul(t1, sfi[0:64, :], c_st[0])
            sog = gates.tile([128, H], bf16, tag="sog", bufs=2)
            nc.scalar.activation(out=sog, in_=pzt["GO"], func=AF.Tanh,
                                 scale=ogsc[:, 0:1])
            # t2 = sig(i) * tanh(g): both operands at partitions 64-127,
            # output written to 0-63 (cross-quadrant write); bf16 2x mode
            t2 = gates.tile([BS, H], bf16, tag="tmp2", bufs=2)
            nc.vector.tensor_mul(t2, sfi[64:128, :], sog[64:128, :])
            c_st[0] = state.tile([BS, H], f32, tag="c", name=f"c_{s}")
            nc.vector.tensor_add(c_st[0], t1, t2)
            tc_sb = gates.tile([BS, H], bf16, tag="tmp2", bufs=2)
            nc.scalar.activation(out=tc_sb, in_=c_st[0], func=AF.Tanh)
            # sigmoid fixup for o (rows 0-63 of sog)
            nc.vector.tensor_scalar(out=sog[0:64, :], in0=sog[0:64, :],
                                    scalar1=0.5, scalar2=0.5,
                                    op0=AL.mult, op1=AL.add)
            hbf[0] = small.tile([BS, H], bf16, tag="h_bf", bufs=2,
                                name=f"h_bf_{s}")
            nc.vector.tensor_mul(hbf[0], sog[0:64, :], tc_sb)

        for s in range(S):
            if s > 0:
                emit_hT_transpose(s)
                qT = emit_qT(s)
                emit_probs(s - 1)
            else:
                qT = None
            pzt = {"FI": pz.tile([128, 512], f32, tag="FI", name=f"pzFI_{s}"),
                   "GO": pz.tile([128, 512], f32, tag="GO", name=f"pzGO_{s}")}
            emit_z_early(s, pzt)
            th = [big.tile([128, 4, T * GB], bf16, tag=f"th{g}", name=f"th{g}_{s}")
                  for g in range(NG)]
            pe_ = [pep.tile([128, 512], f32, tag="pe", name=f"pe{g}_{s}")
                   for g in range(NG)]
            ests = [None] * NG
            aTs = [None] * NG
            reds = [None] * NG
            # group-sequential window: g0's exp/scatter chain hides under
            # g1's tanh half-window. g0's DVE-side post ops (red, parity-1
            # ablk copy) are slotted between g1's broadcast-adds so they
            # never stall the adds feeding ACT.
            for k in range(4):
                emit_chunk(s, 0, k, qT, th[0], pe_[0])
            ests[0], aTs[0] = emit_exp_dma(s, 0, pe_[0])
            emit_chunk(s, 1, 0, qT, th[1], pe_[1])
            reds[0] = emit_red(s, 0, ests[0])
            emit_chunk(s, 1, 1, qT, th[1], pe_[1])
            emit_chunk(s, 1, 2, qT, th[1], pe_[1])
            emit_ablk_copy(0, aTs[0], 1, nc.vector)
            emit_chunk(s, 1, 3, qT, th[1], pe_[1], split=True)
            ests[1], aTs[1] = emit_exp_dma(s, 1, pe_[1])
            reds[1] = emit_red(s, 1, ests[1])
            emit_ablk_copy(1, aTs[1], 1, nc.vector)
            ctx_sb = small.tile([BS, C], bf16, tag="ctx_sb", bufs=2,
                                name=f"ctx_{s}")
            psums = psm.tile([128, 256], f32, tag="pq", name=f"psm_{s}")
            pcE = pcp.tile([128, C], f32, tag="ctxE", name=f"pctxE_{s}")
            pcO = pcp.tile([128, C], f32, tag="ctxO", name=f"pctxO_{s}")
            pxT = [None] * NG
            xTc = small.tile([128, 4, BS], bf16, tag="xTc", bufs=2,
                             name=f"xTc_{s}")
            for g in range(NG):
                csum = small.tile([GB, C], f32, tag=f"csum{g}", bufs=2,
                                  name=f"csum{g}_{s}")
                rcp_t = tiny.tile([GB, 1], f32, tag=f"rcp{g}")
                emit_post(s, g, reds[g], ctx_sb, csum, psums, pcE, pcO, rcp_t)
                pxT[g] = ptp.tile([128, 256], bf16, tag="tp", name=f"pxT{g}_{s}")
                emit_ctxT(s, g, ctx_sb, pxT, xTc)
            emit_z_late(s, pzt, xTc)
            emit_gates(s, pzt)
        emit_hT_transpose(S)
        emit_probs(S - 1)
        dma.dma_start(out=out_d[:], in_=pr_all)

    nc.finalize()
    return nc


def _prep_core(inputs, i):
    bsl = slice(i * BS, (i + 1) * BS)
    bh_i = np.asarray(inputs["batch_H"][bsl], np.float32)          # [64, 64, 512]
    text_i = np.asarray(inputs["text"][bsl])                       # [64, 26]
    bh_g = bh_i.reshape(NG, GB, T, C)
    m = {}
    m["bHT"] = np.ascontiguousarray(bh_g.transpose(0, 3, 2, 1)).astype(BF)
    m["bHc"] = np.ascontiguousarray(bh_g.reshape(NG, GB // 2, 128, C)).astype(BF)
    m["wi"] = np.asarray(inputs["Wi"], np.float32).astype(BF)
    m["wh"] = np.asarray(inputs["Wh"], np.float32).astype(BF)
    m["bh"] = np.ascontiguousarray(
        np.asarray(inputs["bh"], np.float32).reshape(4, 128).T)
    wsr = np.ascontiguousarray(
        np.asarray(inputs["Ws"], np.float32)[:, 0].reshape(4, 128).T).astype(BF)
    m["ws"] = np.repeat(wsr[:, :, None], 32, axis=2)
    lk = np.asarray(inputs["lstm_kernel"], np.float32)
    lb = np.asarray(inputs["lstm_bias"], np.float32)
    m["kc"] = lk[:C].astype(BF)
    m["ko"] = (lk[C:] + lb[None, :]).astype(BF)
    m["rr"] = np.asarray(inputs["lstm_rec"], np.float32).astype(BF)
    m["oh"] = (np.arange(NCC)[:, None, None] == text_i.T[None, :, :]).astype(BF)
    m["wg"] = np.asarray(inputs["Wgen"], np.float32).astype(BF)
    m["bg"] = np.tile(np.asarray(inputs["bgen"], np.float32)[None, :], (BS, 1))
    m["ident"] = np.eye(128, dtype=np.float32).astype(BF)
    return m


def kernel(_trace=False, **inputs):
    from concourse import bass_utils
    if "nc" not in _CACHE:
        _CACHE["nc"] = build_bass()
    nc = _CACHE["nc"]
    in_maps = [_prep_core(inputs, i) for i in range(NCORES)]
    res = bass_utils.run_bass_kernel_spmd(nc, in_maps, list(range(NCORES)),
                                          trace=_trace)
    _CACHE["last_result"] = res
    out = np.concatenate([r["out"] for r in res.results], axis=0)
    return out.astype(np.float32)


# revision 43
# speedup vs baseline: 1.0035x; 1.0035x over previous
"""Bass/Trainium2 kernel for attention-LSTM decoder (nn_Attention_49289044688898).

Data-parallel over batch: 512 rows -> 8 NeuronCores x 64 rows. Weights replicated.
Within a core, 64 rows = 2 groups of 32 for the attention; LSTM/q/probs joint.

Schedule v4 (post-baseline, 1117us -> ~1007us):
- Group-sequential tanh window (g0's 4 chunks then g1's 4) so g0's whole
  post-attention chain (exp -> alphaT -> ablk -> ctx -> transpose) hides
  under g1's tanh half-window; g0's DVE-side post ops are slotted between
  g1's broadcast-adds so they never stall the adds feeding ACT.
- Softmax denominator from est via DVE tl-reduce + ones-matmul over all
  128 partitions (est rows are 32x copies; ones = 1/32 folds that away),
  eliminating the old alphaT->denominator dependency.
- ablk block-diag is NOT DMA'd: per b-parity it is a same-partition
  free-dim restride (stride 2 -> 34) + a +64 partition shift for parity
  1, done as one GPSIMD copy (par 0) and one DVE cross-quadrant copy
  (par 1). Only ONE real DMA remains per group (est row-spread ->
  alphaT, contiguous 64B runs); g1's issues from the ACT queue.
- ctx accumulation alternates two PSUM banks (even/odd kt) so the
  accumulating matmuls pipeline at full stream rate (0.21us vs 0.43us).
- Gates packed: FI bank = [f;i], GO bank = [o;g]; one [128,512] ACT op
  per bank (per-partition scale vector picks tanh(x/2) vs tanh(x)) +
  bf16 DVE fixups/muls (2x mode); c-state stays f32. sig(i)*tanh(g)
  reads partitions 64-127 and writes 0-63 (legal cross-quadrant write).
- g1's last tanh chunk is split in half so its e-mms (and exp) start
  ~1us earlier, shortening the tail.

Per step s (26 steps):
  hT  = transpose(h)                        (PE, 4 transpose-mm)
  qT  = WhT-chunks @ hT                     (PE, 16 mm N=64, k-accum)
  probs(s-1) = hT-mm @ Wgen + bg            (PE + DVE, during tanh window)
  z-partial: onehot@Ko' + h@R               (PE, during tanh window)
  per g, k: th = tanh(HprojT + qT)          (DVE add + ACT tanh)
            e-mms into PSUM quadrants       (PE)
  per g: est = exp(e-PSUM)                  (ACT)
         alphaT via row-spread DMA          (sync / ACT queue)
         ablk via 2 engine copies           (GPSIMD + DVE)
         red[p,b] = sum_tl est              (DVE 3D reduce)
         denom = red^T @ (1/32)             (PE, N=1) -> reciprocal (DVE)
         ctx: E/O bank-alternating mms      (PE), (E+O)*rcp -> ctx_sb (DVE)
  xTc = PE-transpose(ctx)
  z  += xTc @ Kc                            (PE)
  gates: 2 packed ACT ops + fixups + c/h    (ACT+DVE)
Layouts:
  attention world: [128 part = h_lo, 4 h_hi, 64 t, 32 b]
  context world:   [128 part = (b%2)*64 + t, 16 kt=b//2, 512 c]
  LSTM world:      [64 part = b, free]; FI bank=[f;i], GO bank=[o;g]
"""

import numpy as np
import ml_dtypes
from contextlib import ExitStack

B, T, C, H, NCC, S = 512, 64, 512, 512, 96, 26
NCORES = 8
BS = B // NCORES          # 64 batch rows per core
NG = 2                    # groups per core
GB = BS // NG             # 32 rows per group
BF = ml_dtypes.bfloat16

_CACHE = {}


def build_bass():
    import concourse.bass as bass
    import concourse.bacc as bacc
    import concourse.tile as tile
    import concourse.mybir as mybir

    f32 = mybir.dt.float32
    bf16 = mybir.dt.bfloat16
    f8 = mybir.dt.float8e4
    DR = mybir.MatmulPerfMode.DoubleRow
    AF = mybir.ActivationFunctionType
    AX = mybir.AxisListType
    AL = mybir.AluOpType

    nc = bacc.Bacc("TRN2", target_bir_lowering=False)

    # ---- DRAM I/O ----
    bHT_d = nc.dram_tensor("bHT", [NG, C, T, GB], bf16, kind="ExternalInput")
    bHc_d = nc.dram_tensor("bHc", [NG, GB // 2, 128, C], bf16, kind="ExternalInput")
    wi_d = nc.dram_tensor("wi", [C, H], bf16, kind="ExternalInput")
    wh_d = nc.dram_tensor("wh", [H, H], bf16, kind="ExternalInput")
    bh_d = nc.dram_tensor("bh", [128, 4], f32, kind="ExternalInput")
    ws_d = nc.dram_tensor("ws", [128, 4, 32], bf16, kind="ExternalInput")
    kc_d = nc.dram_tensor("kc", [C, 4 * H], bf16, kind="ExternalInput")
    rr_d = nc.dram_tensor("rr", [H, 4 * H], bf16, kind="ExternalInput")
    ko_d = nc.dram_tensor("ko", [NCC, 4 * H], bf16, kind="ExternalInput")
    oh_d = nc.dram_tensor("oh", [NCC, S, BS], bf16, kind="ExternalInput")
    wg_d = nc.dram_tensor("wg", [H, NCC], bf16, kind="ExternalInput")
    bg_d = nc.dram_tensor("bg", [BS, NCC], f32, kind="ExternalInput")
    id_d = nc.dram_tensor("ident", [128, 128], bf16, kind="ExternalInput")
    out_d = nc.dram_tensor("out", [BS, S, NCC], f32, kind="ExternalOutput")

    NCH = T * GB // 512  # 4 (t,b)-chunks of 512 per group

    with tile.TileContext(nc) as tc, ExitStack() as ctx:
        big = ctx.enter_context(tc.tile_pool(name="big", bufs=1))
        wpool = ctx.enter_context(tc.tile_pool(name="wpool", bufs=1))
        small = ctx.enter_context(tc.tile_pool(name="small", bufs=2))
        tiny = ctx.enter_context(tc.tile_pool(name="tiny", bufs=4))
        gates = ctx.enter_context(tc.tile_pool(name="gates", bufs=4))
        state = ctx.enter_context(tc.tile_pool(name="state", bufs=2))
        # PSUM pools (8 banks total):
        #   pz:  FI + GO gate accumulators  [128,512] x2     = 2 banks
        #   pep: e quadrant accumulator     [128,512] bufs=2 = 2 banks
        #   pcp: ctx E/O accumulators       [128,512] x2     = 2 banks
        #   ptp: bf16 PE-transpose outs     [128,256] bufs=1 = 1 bank
        #   psm: qT/probs/sums f32 mm outs  [128,256] bufs=1 = 1 bank
        pz = ctx.enter_context(tc.tile_pool(name="pz", bufs=1, space="PSUM"))
        pep = ctx.enter_context(tc.tile_pool(name="pep", bufs=2, space="PSUM"))
        pcp = ctx.enter_context(tc.tile_pool(name="pcp", bufs=1, space="PSUM"))
        ptp = ctx.enter_context(tc.tile_pool(name="ptp", bufs=1, space="PSUM"))
        psm = ctx.enter_context(tc.tile_pool(name="psm", bufs=1, space="PSUM"))

        dma = nc.sync
        import concourse.bass as _b

        # ---- load weights / big tensors ----
        bHc = [big.tile([128, GB // 2, C], bf16, tag=f"bHc{g}", name=f"bHc{g}")
               for g in range(NG)]
        for g in range(NG):
            dma.dma_start(out=bHc[g], in_=bHc_d[g].rearrange("k p c -> p k c"))
        # batch_H^T (prolog only; shares slots with tanh buffers)
        bHT = [big.tile([128, 4, T * GB], bf16, tag=f"th{g}", name=f"bHT{g}")
               for g in range(NG)]
        for g in range(NG):
            dma.dma_start(
                out=bHT[g],
                in_=bHT_d[g].rearrange("(ch cl) t b -> cl ch (t b)", cl=128))

        wi = wpool.tile([128, 4, H], bf16, tag="wi")
        dma.dma_start(out=wi, in_=wi_d[:].rearrange("(ch cl) h -> cl ch h", cl=128))
        wh = wpool.tile([128, 4, H], bf16, tag="wh")
        dma.dma_start(out=wh, in_=wh_d[:].rearrange("(hh hl) h -> hl hh h", hl=128))
        bh = wpool.tile([128, 4], f32, tag="bh")
        dma.dma_start(out=bh, in_=bh_d[:])
        ws = wpool.tile([128, 4, 32], bf16, tag="ws")
        dma.dma_start(out=ws, in_=ws_d[:])
        kc = wpool.tile([128, 4, 4 * H], bf16, tag="kc")
        dma.dma_start(out=kc, in_=kc_d[:].rearrange("(kh kl) n -> kl kh n", kl=128))
        rr = wpool.tile([128, 4, 4 * H], bf16, tag="rr")
        dma.dma_start(out=rr, in_=rr_d[:].rearrange("(kh kl) n -> kl kh n", kl=128))
        ko = wpool.tile([NCC, 4 * H], bf16, tag="ko")
        dma.dma_start(out=ko, in_=ko_d[:])
        oh = wpool.tile([NCC, S, BS], bf16, tag="oh")
        dma.dma_start(out=oh, in_=oh_d[:])
        wg = wpool.tile([128, 4, NCC], bf16, tag="wg")
        dma.dma_start(out=wg, in_=wg_d[:].rearrange("(hh hl) n -> hl hh n", hl=128))
        bg = wpool.tile([BS, NCC], f32, tag="bg")
        dma.dma_start(out=bg, in_=bg_d[:])
        ident = wpool.tile([128, 128], bf16, tag="ident")
        dma.dma_start(out=ident, in_=id_d[:])
        # 1/32 folds away the x32 copy-redundancy of the est row-blocks, so
        # the ones-matmul over all 128 partitions yields the denominator
        ones32 = wpool.tile([128, 1], bf16, tag="ones32")
        nc.vector.memset(ones32, 1.0 / 32.0)
        # per-partition activation scale for the packed GO bank: o rows
        # (0-63) get 0.5 (sigmoid-via-tanh), g rows (64-127) get 1.0
        ogsc = wpool.tile([128, 1], f32, tag="ogsc")
        nc.vector.memset(ogsc[0:64, :], 0.5)
        nc.vector.memset(ogsc[64:128, :], 1.0)
        pr_all = wpool.tile([BS, S, NCC], f32, tag="pr_all")

        # block-diag alpha holders (zeroed once)
        ablk = [wpool.tile([128, GB // 2, GB], bf16, tag=f"ablk{g}", name=f"ablk{g}")
                for g in range(NG)]
        for g in range(NG):
            nc.vector.memset(ablk[g], 0.0)

        # initial state (hT is never read at s=0: qT/z-rec are skipped)
        hT = [None]
        c_st = [state.tile([BS, H], f32, tag="c", name="c0")]
        nc.vector.memset(c_st[0], 0.0)
        hbf = [None]

        # ---- prolog: HprojT[g] = (batch_H @ Wi)^T + bh ----
        hprojT = [big.tile([128, 4, T * GB], bf16, tag=f"hp{g}", name=f"hp{g}")
                  for g in range(NG)]
        for g in range(NG):
            for m in range(4):
                for n in range(NCH):
                    ps = pz.tile([128, 512], f32, tag="FI" if g == 0 else "GO")
                    for k in range(4):
                        nc.tensor.matmul(
                            ps,
                            wi[:, k, m * 128:(m + 1) * 128],
                            bHT[g][:, k, n * 512:(n + 1) * 512],
                            start=(k == 0), stop=(k == 3),
                        )
                    if (m + n) % 2 == 0:
                        nc.scalar.activation(
                            out=hprojT[g][:, m, n * 512:(n + 1) * 512], in_=ps,
                            func=AF.Identity, bias=bh[:, m:m + 1], scale=1.0,
                        )
                    else:
                        nc.vector.tensor_scalar_add(
                            hprojT[g][:, m, n * 512:(n + 1) * 512], ps,
                            bh[:, m:m + 1])

        def bcast_t(ap2, nt=T):
            # [128, GB(b)] -> [128, nt(t, stride0), GB(b)]
            return _b.AP(tensor=ap2.tensor, offset=ap2.offset,
                         ap=[ap2.ap[0], [0, nt], ap2.ap[1]])

        gate_sl = {"f": 1, "i": 0, "g": 2, "o": 3}
        # gate -> (psum tag, row offset): f/i share FI bank; o/g share GO
        # bank with o LOW (rows 0-63) so h = sig(o)*tanh(c) stays aligned
        # at partitions 0-63 and t2 = sig(i)*tanh(g) reads both at 64-127.
        gate_loc = {"f": ("FI", 0), "i": ("FI", 64), "o": ("GO", 0), "g": ("GO", 64)}

        def emit_hT_transpose(s):
            # h_bf [64, 512] -> hT [128, 4, 64] via 4 PE transposes
            phT = ptp.tile([128, 256], bf16, tag="tp", name=f"phT_{s}")
            for m in range(4):
                nc.tensor.transpose(phT[:, m * 64:(m + 1) * 64],
                                    hbf[0][:, m * 128:(m + 1) * 128],
                                    ident[0:BS, 0:BS])
            hT[0] = state.tile([128, 4, BS], bf16, tag="hT", name=f"hT_{s}")
            nc.vector.tensor_copy(hT[0], phT)

        def emit_qT(s):
            # qT[h',b] = sum_h Wh[h,h'] hT[h,b]; m-outer so chunk m is
            # copied out as soon as its k-accumulation finishes.
            pqT = psm.tile([128, 256], f32, tag="pq", name=f"pqT_{s}")
            qT = small.tile([128, 4, BS], bf16, tag="qT", bufs=2, name=f"qT_{s}")
            for m in range(4):
                for k in range(4):
                    nc.tensor.matmul(pqT[:, m * 64:(m + 1) * 64],
                                     wh[:, k, m * 128:(m + 1) * 128],
                                     hT[0][:, k, :],
                                     start=(k == 0), stop=(k == 3))
                nc.vector.tensor_copy(qT[:, m, :], pqT[:, m * 64:(m + 1) * 64])
            return qT

        def emit_probs(sm1):
            # probs(sm1) = h(sm1) @ Wgen + bg, from hT
            pp = psm.tile([128, 256], f32, tag="pq", name=f"pp_{sm1}")
            for k in range(4):
                nc.tensor.matmul(pp[0:BS, 0:NCC], hT[0][:, k, :], wg[:, k, :],
                                 start=(k == 0), stop=(k == 3))
            nc.vector.tensor_add(pr_all[:, sm1, :], pp[0:BS, 0:NCC], bg)

        def emit_z_early(s, pzt):
            # onehot@Ko' (start) + h@R during the tanh window
            for gn in "figo":
                tag, ro = gate_loc[gn]
                zsl = slice(gate_sl[gn] * 512, (gate_sl[gn] + 1) * 512)
                nc.tensor.matmul(pzt[tag][ro:ro + 64, :], oh[:, s, :],
                                 ko[:, zsl], start=True, stop=False,
                                 tile_position=(0, ro))
            if s == 0:
                return  # h == 0: R contributes nothing
            for k in range(4):
                for gn in "figo":
                    tag, ro = gate_loc[gn]
                    zsl = slice(gate_sl[gn] * 512, (gate_sl[gn] + 1) * 512)
                    nc.tensor.matmul(pzt[tag][ro:ro + 64, :], hT[0][:, k, :],
                                     rr[:, k, zsl], start=False, stop=False,
                                     tile_position=(0, ro))

        def emit_chunk(s, g, k, qT, th_g, pe_g, split=False):
            # DVE add + ACT tanh + 4 e quadrant mms for (group g, chunk k).
            # split=True halves the tanh so the last e-mms (and thus exp)
            # start ~1us earlier — used for the tail-critical (g1, k3).
            gsl_b = slice(g * GB, (g + 1) * GB)
            src = hprojT[g]
            if qT is not None:
                nc.vector.tensor_add(
                    th_g[:, k, :].rearrange("p (t b) -> p t b", t=T),
                    hprojT[g][:, k, :].rearrange("p (t b) -> p t b", t=T),
                    bcast_t(qT[:, k, gsl_b]))
                src = th_g
            halves = 2 if split else 1
            hsz = (T * GB) // halves
            for hv in range(halves):
                sl = slice(hv * hsz, (hv + 1) * hsz)
                nc.scalar.activation(out=th_g[:, k, sl], in_=src[:, k, sl],
                                     func=AF.Tanh)
                for j in range(hv * NCH // halves, (hv + 1) * NCH // halves):
                    bp = 32 * j
                    nc.tensor.matmul(pe_g[bp:bp + 32, :], ws[:, k, :],
                                     th_g[:, k, j * 512:(j + 1) * 512],
                                     start=(k == 0), stop=(k == 3),
                                     tile_position=(0, bp))

        def emit_exp_dma(s, g, pe_g, split=False):
            # est[32j+r, tl*32+b] = exp(e(t=16j+tl, b)) for all r (the e-mm
            # wrote 32 identical copies per j). alphaT[t, b] built with ONE
            # row-spread DMA (contiguous 64B runs). The ablk block-diag is
            # NOT a DMA: per parity it is a same-partition free-dim restride
            # (stride 2 -> 34) plus a +64 partition shift for parity 1, so
            # parity 0 goes on idle GPSIMD and parity 1 on DVE (legal
            # cross-quadrant write for a 64-wide op).
            est = small.tile([128, 512], bf16, tag=f"est{g}", bufs=1,
                             name=f"est{g}_{s}")
            nc.scalar.activation(out=est, in_=pe_g, func=AF.Exp)
            ea = est[:]
            pe0 = ea.ap[0][0]
            alphaT = small.tile([T, GB], bf16, tag=f"alphaT{g}", bufs=2,
                                name=f"alphaT{g}_{s}")
            src = _b.AP(tensor=ea.tensor, offset=ea.offset,
                        ap=[[32 * pe0, 4], [32, 16], [1, 32]])
            # g1's spread DMA issues from the ACT queue right after its exp
            # (saves a cross-engine semaphore hop; ACT is idle post-window)
            (nc.scalar if g == 1 else dma).dma_start(out=alphaT[:, :], in_=src)
            emit_ablk_copy(g, alphaT, 0, nc.gpsimd)
            return est, alphaT

        def emit_ablk_copy(g, alphaT, par, eng):
            at = alphaT[:]
            ab = ablk[g][:]
            srcp = _b.AP(tensor=at.tensor,
                         offset=at.offset + par * at.ap[1][0],
                         ap=[[at.ap[0][0], T], [2 * at.ap[1][0], GB // 2]])
            dst = _b.AP(tensor=ab.tensor,
                        offset=ab.offset + par * (64 * ab.ap[0][0] + ab.ap[2][0]),
                        ap=[[ab.ap[0][0], T], [ab.ap[1][0] + 2 * ab.ap[2][0], GB // 2]])
            eng.tensor_copy(dst, srcp)

        def emit_red(s, g, est):
            # red[p, b] = sum_tl est[p, tl*32+b]; all 128 rows valid, so the
            # later ones-matmul over partitions yields 32x the denominator.
            red = small.tile([128, GB], bf16, tag=f"red{g}", bufs=2,
                             name=f"red{g}_{s}")
            with nc.allow_low_precision(reason="16-term positive sums; bf16 ok"):
                nc.vector.tensor_reduce(
                    out=red, in_=est[:].rearrange("p (tl b) -> p b tl", b=GB),
                    axis=AX.X, op=AL.add)
            return red

        def emit_post(s, g, red, ctx_sb, csum, psums, pcE, pcO, rcp_t):
            # 32*denominator via ones-matmul over partitions (per-group free
            # column of psums); ctx mms alternate E/O PSUM banks, both
            # groups on PSUM rows 0-31 (DoubleRow requires dst partition 0).
            ro = g * GB
            nc.tensor.matmul(psums[0:GB, g:g + 1], red, ones32,
                             start=True, stop=True, tile_position=(0, 0))
            nc.vector.reciprocal(rcp_t, psums[0:GB, g:g + 1])
            # kt-mms alternate E/O banks so accumulation pipelines
            banks = [pcE, pcO]
            for kt in range(GB // 2):
                nc.tensor.matmul(banks[kt % 2][0:GB, :], ablk[g][:, kt, :],
                                 bHc[g][:, kt, :],
                                 start=(kt < 2), stop=(kt >= GB // 2 - 2),
                                 tile_position=(0, 0))
            # (E + O) / denom with only one PSUM operand per DVE op; the
            # final op's out for g1 is a legal cross-quadrant 32-wide write
            nc.vector.tensor_scalar_mul(csum, pcE[0:GB, :], rcp_t)
            nc.vector.scalar_tensor_tensor(
                out=ctx_sb[ro:ro + GB, :], in0=pcO[0:GB, :],
                scalar=rcp_t, in1=csum,
                op0=AL.mult, op1=AL.add)

        def emit_ctxT(s, g, ctx_sb, pxT, xTc):
            # ctx rows of group g -> xTc[:, k, g*32:(g+1)*32]
            ro = g * GB
            for k in range(4):
                nc.tensor.transpose(pxT[g][:, k * GB:(k + 1) * GB],
                                    ctx_sb[ro:ro + GB, k * 128:(k + 1) * 128],
                                    ident[ro:ro + GB, ro:ro + GB])
            src = pxT[g][:, 0:128].rearrange("p (k b) -> p k b", k=4)
            nc.vector.tensor_copy(xTc[:, :, g * GB:(g + 1) * GB], src)

        def emit_z_late(s, pzt, xTc):
            for pair in ("fi", "og"):
                for k in range(4):
                    for gn in pair:
                        tag, ro = gate_loc[gn]
                        zsl = slice(gate_sl[gn] * 512, (gate_sl[gn] + 1) * 512)
                        nc.tensor.matmul(pzt[tag][ro:ro + 64, :], xTc[:, k, :],
                                         kc[:, k, zsl], start=False,
                                         stop=(k == 3), tile_position=(0, ro))

        def emit_gates(s, pzt):
            # FI bank = [f 0-63; i 64-127]: one tanh(x/2) op + DVE fixup.
            # GO bank = [o 0-63; g 64-127]: one tanh op with per-partition
            # scale (0.5 for o, 1.0 for g) + DVE fixup on the o half only.
            sfi = gates.tile([128, H], bf16, tag="sfi", bufs=2)
            nc.scalar.activation(out=sfi, in_=pzt["FI"], func=AF.Tanh, scale=0.5)
            nc.vector.tensor_scalar(out=sfi, in0=sfi, scalar1=0.5, scalar2=0.5,
                                    op0=AL.mult, op1=AL.add)
            # t1 first: it only needs the FI fixup, not sog
            t1 = gates.tile([BS, H], f32, tag="tmp", bufs=2)
            nc.vector.tensor_mul(t1, sfi[0:64, :], c_st[0])
            sog = gates.tile([128, H], bf16, tag="sog", bufs=2)
            nc.scalar.activation(out=sog, in_=pzt["GO"], func=AF.Tanh,
                                 scale=ogsc[:, 0:1])
            # t2 = sig(i) * tanh(g): both operands at partitions 64-127,
            # output written to 0-63 (cross-quadrant write); bf16 2x mode
            t2 = gates.tile([BS, H], bf16, tag="tmp2", bufs=2)
            nc.vector.tensor_mul(t2, sfi[64:128, :], sog[64:128, :])
            c_st[0] = state.tile([BS, H], f32, tag="c", name=f"c_{s}")
            nc.vector.tensor_add(c_st[0], t1, t2)
            tc_sb = gates.tile([BS, H], bf16, tag="tmp2", bufs=2)
            nc.scalar.activation(out=tc_sb, in_=c_st[0], func=AF.Tanh)
            # sigmoid fixup for o (rows 0-63 of sog)
            nc.vector.tensor_scalar(out=sog[0:64, :], in0=sog[0:64, :],
                                    scalar1=0.5, scalar2=0.5,
                                    op0=AL.mult, op1=AL.add)
            hbf[0] = small.tile([BS, H], bf16, tag="h_bf", bufs=2,
                                name=f"h_bf_{s}")
            nc.vector.tensor_mul(hbf[0], sog[0:64, :], tc_sb)

        for s in range(S):
            if s > 0:
                emit_hT_transpose(s)
                qT = emit_qT(s)
                emit_probs(s - 1)
            else:
                qT = None
            pzt = {"FI": pz.tile([128, 512], f32, tag="FI", name=f"pzFI_{s}"),
                   "GO": pz.tile([128, 512], f32, tag="GO", name=f"pzGO_{s}")}
            emit_z_early(s, pzt)
            th = [big.tile([128, 4, T * GB], bf16, tag=f"th{g}", name=f"th{g}_{s}")
                  for g in range(NG)]
            pe_ = [pep.tile([128, 512], f32, tag="pe", name=f"pe{g}_{s}")
                   for g in range(NG)]
            ests = [None] * NG
            aTs = [None] * NG
            reds = [None] * NG
            # group-sequential window: g0's exp/scatter chain hides under
            # g1's tanh half-window. g0's DVE-side post ops (red, parity-1
            # ablk copy) are slotted between g1's broadcast-adds so they
            # never stall the adds feeding ACT.
            for k in range(4):
                emit_chunk(s, 0, k, qT, th[0], pe_[0])
            ests[0], aTs[0] = emit_exp_dma(s, 0, pe_[0])
            emit_chunk(s, 1, 0, qT, th[1], pe_[1])
            reds[0] = emit_red(s, 0, ests[0])
            emit_chunk(s, 1, 1, qT, th[1], pe_[1])
            emit_chunk(s, 1, 2, qT, th[1], pe_[1])
            emit_ablk_copy(0, aTs[0], 1, nc.vector)
            emit_chunk(s, 1, 3, qT, th[1], pe_[1], split=True)
            ests[1], aTs[1] = emit_exp_dma(s, 1, pe_[1])
            reds[1] = emit_red(s, 1, ests[1])
            emit_ablk_copy(1, aTs[1], 1, nc.vector)
            ctx_sb = small.tile([BS, C], bf16, tag="ctx_sb", bufs=2,
                                name=f"ctx_{s}")
            psums = psm.tile([128, 256], f32, tag="pq", name=f"psm_{s}")
            pcE = pcp.tile([128, C], f32, tag="ctxE", name=f"pctxE_{s}")
            pcO = pcp.tile([128, C], f32, tag="ctxO", name=f"pctxO_{s}")
            pxT = [None] * NG
            xTc = small.tile([128, 4, BS], bf16, tag="xTc", bufs=2,
                             name=f"xTc_{s}")
            for g in range(NG):
                csum = small.tile([GB, C], f32, tag=f"csum{g}", bufs=2,
                                  name=f"csum{g}_{s}")
                rcp_t = tiny.tile([GB, 1], f32, tag=f"rcp{g}")
                emit_post(s, g, reds[g], ctx_sb, csum, psums, pcE, pcO, rcp_t)
                pxT[g] = ptp.tile([128, 256], bf16, tag="tp", name=f"pxT{g}_{s}")
                emit_ctxT(s, g, ctx_sb, pxT, xTc)
            emit_z_late(s, pzt, xTc)
            emit_gates(s, pzt)
        emit_hT_transpose(S)
        emit_probs(S - 1)
        dma.dma_start(out=out_d[:], in_=pr_all)

    nc.finalize()
    return nc


def _prep_core(inputs, i):
    bsl = slice(i * BS, (i + 1) * BS)
    bh_i = np.asarray(inputs["batch_H"][bsl], np.float32)          # [64, 64, 512]
    text_i = np.asarray(inputs["text"][bsl])                       # [64, 26]
    bh_g = bh_i.reshape(NG, GB, T, C)
    m = {}
    m["bHT"] = np.ascontiguousarray(bh_g.transpose(0, 3, 2, 1)).astype(BF)
    m["bHc"] = np.ascontiguousarray(bh_g.reshape(NG, GB // 2, 128, C)).astype(BF)
    m["wi"] = np.asarray(inputs["Wi"], np.float32).astype(BF)
    m["wh"] = np.asarray(inputs["Wh"], np.float32).astype(BF)
    m["bh"] = np.ascontiguousarray(
        np.asarray(inputs["bh"], np.float32).reshape(4, 128).T)
    wsr = np.ascontiguousarray(
        np.asarray(inputs["Ws"], np.float32)[:, 0].reshape(4, 128).T).astype(BF)
    m["ws"] = np.repeat(wsr[:, :, None], 32, axis=2)
    lk = np.asarray(inputs["lstm_kernel"], np.float32)
    lb = np.asarray(inputs["lstm_bias"], np.float32)
    m["kc"] = lk[:C].astype(BF)
    m["ko"] = (lk[C:] + lb[None, :]).astype(BF)
    m["rr"] = np.asarray(inputs["lstm_rec"], np.float32).astype(BF)
    m["oh"] = (np.arange(NCC)[:, None, None] == text_i.T[None, :, :]).astype(BF)
    m["wg"] = np.asarray(inputs["Wgen"], np.float32).astype(BF)
    m["bg"] = np.tile(np.asarray(inputs["bgen"], np.float32)[None, :], (BS, 1))
    m["ident"] = np.eye(128, dtype=np.float32).astype(BF)
    return m


def kernel(_trace=False, **inputs):
    from concourse import bass_utils
    if "nc" not in _CACHE:
        _CACHE["nc"] = build_bass()
    nc = _CACHE["nc"]
    in_maps = [_prep_core(inputs, i) for i in range(NCORES)]
    res = bass_utils.run_bass_kernel_spmd(nc, in_maps, list(range(NCORES)),
                                          trace=_trace)
    _CACHE["last_result"] = res
    out = np.concatenate([r["out"] for r in res.results], axis=0)
    return out.astype(np.float32)
